# revision 3
# baseline (speedup 1.0000x reference)
"""GRNN regressor on 8 TRN2 NeuronCores.

Math: w[b,n] = exp(-(||x_b||^2 + ||t_n||^2 - 2 x_b.t_n)/2); out[b] = (w@y)/(w@1).

Strategy (matches the sharding hint): X_train/y_train sharded over N across
8 cores; x replicated. Per core, one matmul with an augmented feature dim
(K=66: 64 features + both squared-norm terms) produces -dist^2/2 directly in
PSUM with train-points on partitions; ScalarE Exp turns it into bf16 weights;
a second PSUM-accumulating matmul against [y, 1] contracts over train points,
yielding per-core partial [numerator; denominator] of shape [2, 4096].
The all-reduce over cores plus the final divide happen on host (32KB/core).

Perf notes vs the naive version:
- float32r matmul dtype: 1 cycle/row at moving-dim 512 (plain fp32 is 4).
- Exp output and the [y,1] operand are bf16 so the second matmul also runs
  at 1 cycle/row.
- Activations are batched over G=3 PSUM banks (free dim 1536) to amortize
  the ~185ns fixed Act-engine access latency per instruction.
"""

import numpy as np

B, F, N, P = 4096, 64, 100000, 8
NS = N // P            # 12500 train points per core
NB = 128               # train-point block (PSUM partitions)
NSP = ((NS + NB - 1) // NB) * NB   # 12544 padded
NBLK = NSP // NB       # 98
BBLK = 512             # query block (moving free dim / PSUM bank)
K = F + 2              # augmented contraction dim
G = 3                  # PSUM banks per activation batch

_cache = {}


def _build(reps=1):
    import concourse.bacc as bacc
    import concourse.mybir as mybir
    import concourse.tile as tile

    f32 = mybir.dt.float32
    f32r = mybir.dt.float32r
    bf16 = mybir.dt.bfloat16
    nc = bacc.Bacc("TRN2", target_bir_lowering=False, debug=False)
    xa_d = nc.dram_tensor("xa", [K, B], f32r, kind="ExternalInput")
    ta_d = nc.dram_tensor("ta", [K, NSP], f32r, kind="ExternalInput")
    y1_d = nc.dram_tensor("y1", [NB, 2 * NBLK], bf16, kind="ExternalInput")
    out_d = nc.dram_tensor("out", [2, B], f32, kind="ExternalOutput")

    with tile.TileContext(nc) as tc:
        with (
            tc.tile_pool(name="const", bufs=1) as cpool,
            tc.tile_pool(name="w", bufs=3) as wpool,
            tc.tile_pool(name="res", bufs=2) as rpool,
            tc.tile_pool(name="ps", bufs=2, space="PSUM") as spool,
            tc.tile_pool(name="pacc", bufs=2, space="PSUM") as apool,
        ):
            xa = cpool.tile([K, B], f32r)
            ta = cpool.tile([K, NSP], f32r)
            y1 = cpool.tile([NB, 2 * NBLK], bf16)
            nc.sync.dma_start(xa[:], xa_d[:])
            nc.sync.dma_start(ta[:], ta_d[:])
            nc.sync.dma_start(y1[:], y1_d[:])

            groups = []
            ni = 0
            while ni < NBLK:
                g = min(G, NBLK - ni)
                groups.append((ni, g))
                ni += g

            # Software pipeline: the accumulating matmul for group i is
            # emitted after group i+1's dist matmuls, so the Tensor engine
            # never stalls in-order behind an Exp it depends on.
            def emit_mm2(acc, w, ni, g):
                for j in range(g):
                    nc.tensor.matmul(
                        acc[:],
                        y1[:, 2 * (ni + j) : 2 * (ni + j) + 2],
                        w[:, j * BBLK : (j + 1) * BBLK],
                        start=(ni + j == 0), stop=(ni + j == NBLK - 1),
                    )

            accs = {}
            pend = None
            for r in range(reps):
                for b in range(B // BBLK):
                    acc = apool.tile([2, BBLK], f32)
                    accs[(r, b)] = acc
                    xsl = xa[:, b * BBLK : (b + 1) * BBLK]
                    for ni, g in groups:
                        s = spool.tile([NB, G * BBLK], f32)
                        for j in range(g):
                            nc.tensor.matmul(
                                s[:, j * BBLK : (j + 1) * BBLK],
                                ta[:, (ni + j) * NB : (ni + j + 1) * NB],
                                xsl,
                                start=True, stop=True,
                            )
                        w = wpool.tile([NB, G * BBLK], bf16)
                        nc.scalar.activation(
                            w[:, : g * BBLK], s[:, : g * BBLK],
                            mybir.ActivationFunctionType.Exp,
                        )
                        if pend is not None:
                            emit_mm2(*pend)
                        pend = (acc, w, ni, g)
                    # drain the previous block's finished accumulator
                    if b > 0 or r > 0:
                        pb = (r, b - 1) if b > 0 else (r - 1, B // BBLK - 1)
                        res = rpool.tile([2, BBLK], f32)
                        nc.vector.tensor_copy(res[:], accs.pop(pb)[:])
                        nc.sync.dma_start(
                            out_d[:, pb[1] * BBLK : (pb[1] + 1) * BBLK], res[:]
                        )
            emit_mm2(*pend)
            pend = None
            pb = (reps - 1, B // BBLK - 1)
            res = rpool.tile([2, BBLK], f32)
            nc.vector.tensor_copy(res[:], accs.pop(pb)[:])
            nc.sync.dma_start(out_d[:, pb[1] * BBLK : (pb[1] + 1) * BBLK], res[:])

    nc.compile()
    return nc


def _prep_inputs(x, X_train, y_train):
    import ml_dtypes

    x = np.asarray(x, np.float32)
    X_train = np.asarray(X_train, np.float32)
    y_train = np.asarray(y_train, np.float32)

    xa = np.empty((K, B), np.float32)
    xa[:F] = x.T
    xa[F] = -0.5 * np.sum(x * x, axis=1)
    xa[F + 1] = 1.0

    in_maps = []
    for c in range(P):
        t = X_train[c * NS : (c + 1) * NS]
        ta = np.zeros((K, NSP), np.float32)
        ta[:F, :NS] = t.T
        ta[F, :] = 1.0
        ta[F + 1, :NS] = -0.5 * np.sum(t * t, axis=1)
        ta[F + 1, NS:] = -1e30  # pad columns get weight exp(-inf) = 0
        y1 = np.zeros((NB, 2 * NBLK), ml_dtypes.bfloat16)
        yc = np.zeros(NSP, np.float32)
        yc[:NS] = y_train[c * NS : (c + 1) * NS]
        y1[:, 0::2] = yc.reshape(NBLK, NB).T.astype(ml_dtypes.bfloat16)
        y1[:, 1::2] = 1.0
        in_maps.append({"xa": xa, "ta": ta, "y1": y1})
    return in_maps


def kernel(x, X_train, y_train):
    from concourse.bass_utils import run_bass_kernel_spmd

    in_maps = _prep_inputs(x, X_train, y_train)
    if "nc" not in _cache:
        _cache["nc"] = _build()
    res = run_bass_kernel_spmd(_cache["nc"], in_maps, core_ids=list(range(P)))
    parts = np.stack([np.asarray(r["out"]) for r in res.results])  # [P, 2, B]
    tot = parts.sum(axis=0, dtype=np.float64)
    return (tot[0] / tot[1]).astype(np.float32)


# revision 15
# speedup vs baseline: 2.0139x; 2.0139x over previous
"""GRNN regressor on 8 TRN2 NeuronCores.

Math: w[b,n] = exp(-(||x_b||^2 + ||t_n||^2 - 2 x_b.t_n)/2); out[b] = (w@y)/(w@1).

Strategy (matches the sharding hint): X_train/y_train sharded over N across
8 cores; x replicated. Per core, one matmul with an augmented feature dim
(K=66: 64 features + both squared-norm terms) produces -dist^2/2 directly in
PSUM with train-points on partitions; ScalarE Exp turns it into bf16 weights;
a second PSUM-accumulating matmul against [y, 1] contracts over train points,
yielding per-core partial [numerator; denominator] of shape [2, 4096].
The all-reduce over cores plus the final divide happen on host (32KB/core).

Perf notes vs the naive version:
- float32r matmul dtype: 1 cycle/row at moving-dim 512 (plain fp32 is 4).
- Exp output and the [y,1] operand are bf16 so the second matmul also runs
  at 1 cycle/row.
- Activations are batched over G=3 PSUM banks (free dim 1536) to amortize
  the ~185ns fixed Act-engine access latency per instruction.
"""

import numpy as np

B, F, N, P = 4096, 64, 100000, 8
NS = N // P            # 12500 train points per core
NB = 128               # train-point block (PSUM partitions)
NSP = ((NS + NB - 1) // NB) * NB   # 12544 padded
NBLK = NSP // NB       # 98
BBLK = 512             # query block (moving free dim / PSUM bank)
K = F + 2              # augmented contraction dim
G = 3                  # PSUM banks per activation batch

_cache = {}


def _build(reps=1, stages=("mm1", "act", "mm2"), mmdt="f32r", actmode="exp"):
    import concourse.bacc as bacc
    import concourse.mybir as mybir
    import concourse.tile as tile

    f32 = mybir.dt.float32
    f32r = mybir.dt.float32r
    bf16 = mybir.dt.bfloat16
    mdt = {"f32r": f32r, "f32": f32, "bf16": bf16}[mmdt]
    nc = bacc.Bacc("TRN2", target_bir_lowering=False, debug=False)
    xa_d = nc.dram_tensor("xa", [K, B], mdt, kind="ExternalInput")
    ta_d = nc.dram_tensor("ta", [K, NSP], mdt, kind="ExternalInput")
    y1_d = nc.dram_tensor("y1", [NB, 2 * NBLK], bf16, kind="ExternalInput")
    out_d = nc.dram_tensor("out", [2, B], f32, kind="ExternalOutput")

    with tile.TileContext(nc) as tc:
        with (
            tc.tile_pool(name="const", bufs=1) as cpool,
            tc.tile_pool(name="w", bufs=3) as wpool,
            tc.tile_pool(name="res", bufs=2) as rpool,
            tc.tile_pool(name="ps", bufs=2, space="PSUM") as spool,
            tc.tile_pool(name="pacc", bufs=2, space="PSUM") as apool,
        ):
            xa = cpool.tile([K, B], mdt)
            ta = cpool.tile([K, NSP], mdt)
            y1 = cpool.tile([NB, 2 * NBLK], bf16)
            nc.sync.dma_start(xa[:], xa_d[:])
            nc.sync.dma_start(ta[:], ta_d[:])
            nc.sync.dma_start(y1[:], y1_d[:])

            groups = []
            ni = 0
            while ni < NBLK:
                g = min(G, NBLK - ni)
                groups.append((ni, g))
                ni += g

            # Software pipeline: the accumulating matmul for group i is
            # emitted after group i+1's dist matmuls, so the Tensor engine
            # never stalls in-order behind an Exp it depends on.
            def emit_mm2(acc, w, ni, g):
                if "mm2" not in stages:
                    return
                for j in range(g):
                    nc.tensor.matmul(
                        acc[:],
                        y1[:, 2 * (ni + j) : 2 * (ni + j) + 2],
                        w[:, j * BBLK : (j + 1) * BBLK],
                        start=(ni + j == 0), stop=(ni + j == NBLK - 1),
                    )

            accs = {}
            pend = None
            for r in range(reps):
                for b in range(B // BBLK):
                    acc = apool.tile([2, BBLK], f32)
                    accs[(r, b)] = acc
                    xsl = xa[:, b * BBLK : (b + 1) * BBLK]
                    for ni, g in groups:
                        s = spool.tile([NB, G * BBLK], f32)
                        if "mm1" in stages:
                            for j in range(g):
                                nc.tensor.matmul(
                                    s[:, j * BBLK : (j + 1) * BBLK],
                                    ta[:, (ni + j) * NB : (ni + j + 1) * NB],
                                    xsl,
                                    start=True, stop=True,
                                )
                        w = wpool.tile([NB, G * BBLK], bf16)
                        if "act" in stages:
                            if actmode == "exp":
                                nc.scalar.activation(
                                    w[:, : g * BBLK], s[:, : g * BBLK],
                                    mybir.ActivationFunctionType.Exp,
                                )
                            else:
                                nc.vector.tensor_copy(
                                    w[:, : g * BBLK], s[:, : g * BBLK]
                                )
                        if pend is not None:
                            emit_mm2(*pend)
                        pend = (acc, w, ni, g)
                    # drain the previous block's finished accumulator
                    if b > 0 or r > 0:
                        pb = (r, b - 1) if b > 0 else (r - 1, B // BBLK - 1)
                        res = rpool.tile([2, BBLK], f32)
                        if "mm2" in stages:
                            nc.vector.tensor_copy(res[:], accs.pop(pb)[:])
                        else:
                            accs.pop(pb)
                            nc.vector.memset(res[:], 0.0)
                        nc.sync.dma_start(
                            out_d[:, pb[1] * BBLK : (pb[1] + 1) * BBLK], res[:]
                        )
            emit_mm2(*pend)
            pend = None
            pb = (reps - 1, B // BBLK - 1)
            res = rpool.tile([2, BBLK], f32)
            if "mm2" in stages:
                nc.vector.tensor_copy(res[:], accs.pop(pb)[:])
            else:
                accs.pop(pb)
                nc.vector.memset(res[:], 0.0)
            nc.sync.dma_start(out_d[:, pb[1] * BBLK : (pb[1] + 1) * BBLK], res[:])

    nc.compile()
    return nc


def _build_samestat(reps=1, rb=3, mmdt="f32r"):
    """Stationary-reuse variant: one ta-block stationary serves `rb`
    consecutive query blocks; per-super accumulators are packed into a
    single PSUM bank as [2*rb, 512] via zero-padded [128, 2*rb] y
    stationaries."""
    import concourse.bacc as bacc
    import concourse.mybir as mybir
    import concourse.tile as tile

    f32 = mybir.dt.float32
    f32r = mybir.dt.float32r
    bf16 = mybir.dt.bfloat16
    mdt = {"f32r": f32r, "f32": f32, "bf16": bf16}[mmdt]
    nc = bacc.Bacc("TRN2", target_bir_lowering=False, debug=False)
    xa_d = nc.dram_tensor("xa", [K, B], mdt, kind="ExternalInput")
    ta_d = nc.dram_tensor("ta", [K, NSP], mdt, kind="ExternalInput")
    y1z_d = nc.dram_tensor(
        "y1z", [NB, NBLK * 3 * 6], bf16, kind="ExternalInput"
    )
    out_d = nc.dram_tensor("out", [2, B], f32, kind="ExternalOutput")

    nb = B // BBLK
    supers = [list(range(i, min(i + rb, nb))) for i in range(0, nb, rb)]

    with tile.TileContext(nc) as tc:
        with (
            tc.tile_pool(name="const", bufs=1) as cpool,
            tc.tile_pool(name="w", bufs=3) as wpool,
            tc.tile_pool(name="res", bufs=2) as rpool,
            tc.tile_pool(name="ps", bufs=2, space="PSUM") as spool,
            tc.tile_pool(name="pacc", bufs=2, space="PSUM") as apool,
        ):
            xa = cpool.tile([K, B], mdt)
            ta = cpool.tile([K, NSP], mdt)
            y1z = cpool.tile([NB, NBLK * 3 * 6], bf16)
            nc.sync.dma_start(xa[:], xa_d[:])
            nc.sync.dma_start(ta[:], ta_d[:])
            nc.sync.dma_start(y1z[:], y1z_d[:])

            def emit_mm2(acc, w, ni, blocks):
                for i in range(len(blocks)):
                    c0 = (ni * 3 + i) * 6
                    nc.tensor.matmul(
                        acc[:],
                        y1z[:, c0 : c0 + 6],
                        w[:, i * BBLK : (i + 1) * BBLK],
                        start=(ni == 0 and i == 0),
                        stop=(ni == NBLK - 1 and i == len(blocks) - 1),
                    )

            def drain(acc, blocks):
                res = rpool.tile([6, BBLK], f32)
                nc.vector.tensor_copy(res[:], acc[:])
                for i, b in enumerate(blocks):
                    nc.sync.dma_start(
                        out_d[:, b * BBLK : (b + 1) * BBLK],
                        res[2 * i : 2 * i + 2, :],
                    )

            pend = None
            done = None
            for r in range(reps):
                for blocks in supers:
                    acc = apool.tile([6, BBLK], f32)
                    for ni in range(NBLK):
                        s = spool.tile([NB, 3 * BBLK], f32)
                        for i, b in enumerate(blocks):
                            nc.tensor.matmul(
                                s[:, i * BBLK : (i + 1) * BBLK],
                                ta[:, ni * NB : (ni + 1) * NB],
                                xa[:, b * BBLK : (b + 1) * BBLK],
                                start=True, stop=True,
                            )
                        w = wpool.tile([NB, 3 * BBLK], bf16)
                        nc.scalar.activation(
                            w[:, : len(blocks) * BBLK],
                            s[:, : len(blocks) * BBLK],
                            mybir.ActivationFunctionType.Exp,
                        )
                        if pend is not None:
                            emit_mm2(*pend)
                            if done is not None:
                                drain(*done)
                                done = None
                        pend = (acc, w, ni, blocks)
                    done = (acc, blocks)
            emit_mm2(*pend)
            drain(*done)

    nc.compile()
    return nc


def _build_v3(reps=1, rb=3, use_bias=True, actsplit=False):
    """Split-bf16 kernel: logit[n,b] = x_hi.(t_hi+t_lo) + bias_n, with
    bias_n = -||t_n||^2/2 folded into the Exp activation's per-partition
    bias; the query-norm factor cancels in numerator/denominator. One
    K=128 bf16 matmul per (train-block, query-block), stationary reused
    across `rb` query blocks; packed [6,512] PSUM accumulators."""
    import concourse.bacc as bacc
    import concourse.mybir as mybir
    import concourse.tile as tile

    f32 = mybir.dt.float32
    bf16 = mybir.dt.bfloat16
    nc = bacc.Bacc("TRN2", target_bir_lowering=False, debug=False)
    xa_d = nc.dram_tensor("xa2", [NB, B], bf16, kind="ExternalInput")
    ta_d = nc.dram_tensor("ta2", [NB, NSP], bf16, kind="ExternalInput")
    bi_d = nc.dram_tensor("bias_t", [NB, NBLK], f32, kind="ExternalInput")
    y1z_d = nc.dram_tensor(
        "y1z", [NB, NBLK * 3 * 6], bf16, kind="ExternalInput"
    )
    out_d = nc.dram_tensor("out", [2, B], f32, kind="ExternalOutput")

    nb = B // BBLK
    supers = [list(range(i, min(i + rb, nb))) for i in range(0, nb, rb)]

    with tile.TileContext(nc) as tc:
        with (
            tc.tile_pool(name="const", bufs=1) as cpool,
            tc.tile_pool(name="w", bufs=3) as wpool,
            tc.tile_pool(name="res", bufs=2) as rpool,
            tc.tile_pool(name="ps", bufs=2, space="PSUM") as spool,
            tc.tile_pool(name="pacc", bufs=2, space="PSUM") as apool,
        ):
            xa = cpool.tile([NB, B], bf16)
            ta = cpool.tile([NB, NSP], bf16)
            bi = cpool.tile([NB, NBLK], f32)
            y1z = cpool.tile([NB, NBLK * 3 * 6], bf16)
            nc.sync.dma_start(xa[:], xa_d[:])
            nc.sync.dma_start(ta[:], ta_d[:])
            nc.sync.dma_start(bi[:], bi_d[:])
            nc.sync.dma_start(y1z[:], y1z_d[:])

            def emit_mm2(acc, w, ni, blocks):
                for i in range(len(blocks)):
                    c0 = (ni * 3 + i) * 6
                    nc.tensor.matmul(
                        acc[:],
                        y1z[:, c0 : c0 + 6],
                        w[:, i * BBLK : (i + 1) * BBLK],
                        start=(ni == 0 and i == 0),
                        stop=(ni == NBLK - 1 and i == len(blocks) - 1),
                    )

            def drain(acc, blocks):
                res = rpool.tile([6, BBLK], f32)
                nc.vector.tensor_copy(res[:], acc[:])
                for i, b in enumerate(blocks):
                    nc.sync.dma_start(
                        out_d[:, b * BBLK : (b + 1) * BBLK],
                        res[2 * i : 2 * i + 2, :],
                    )

            pend = None
            done = None
            for r in range(reps):
                for blocks in supers:
                    acc = apool.tile([6, BBLK], f32)
                    for ni in range(NBLK):
                        s = spool.tile([NB, 3 * BBLK], f32)
                        for i, b in enumerate(blocks):
                            nc.tensor.matmul(
                                s[:, i * BBLK : (i + 1) * BBLK],
                                ta[:, ni * NB : (ni + 1) * NB],
                                xa[:, b * BBLK : (b + 1) * BBLK],
                                start=True, stop=True,
                            )
                        w = wpool.tile([NB, 3 * BBLK], bf16)
                        if actsplit:
                            for i in range(len(blocks)):
                                nc.scalar.activation(
                                    w[:, i * BBLK : (i + 1) * BBLK],
                                    s[:, i * BBLK : (i + 1) * BBLK],
                                    mybir.ActivationFunctionType.Exp,
                                    bias=bi[:, ni : ni + 1] if use_bias else 0.0,
                                )
                        else:
                            nc.scalar.activation(
                                w[:, : len(blocks) * BBLK],
                                s[:, : len(blocks) * BBLK],
                                mybir.ActivationFunctionType.Exp,
                                bias=bi[:, ni : ni + 1] if use_bias else 0.0,
                            )
                        if pend is not None:
                            emit_mm2(*pend)
                            if done is not None:
                                drain(*done)
                                done = None
                        pend = (acc, w, ni, blocks)
                    done = (acc, blocks)
            emit_mm2(*pend)
            drain(*done)

    nc.compile()
    return nc


def _prep_inputs_v3(x, X_train, y_train):
    import ml_dtypes

    bf = ml_dtypes.bfloat16
    x = np.asarray(x, np.float32)
    X_train = np.asarray(X_train, np.float32)
    y_train = np.asarray(y_train, np.float32)

    x_hi = x.astype(bf).astype(np.float32)
    xa2 = np.empty((NB, B), bf)
    xa2[:F] = x_hi.T
    xa2[F:] = x_hi.T

    in_maps = []
    for c in range(P):
        t = X_train[c * NS : (c + 1) * NS]
        t_hi = t.astype(bf).astype(np.float32)
        t_lo = (t - t_hi).astype(bf).astype(np.float32)
        t2 = t_hi + t_lo
        ta2 = np.zeros((NB, NSP), bf)
        ta2[:F, :NS] = t_hi.T
        ta2[F:, :NS] = t_lo.T
        bias_t = np.full(NSP, -1e30, np.float32)
        bias_t[:NS] = -0.5 * np.sum(t2 * t2, axis=1)
        bias_t = np.ascontiguousarray(
            bias_t.reshape(NBLK, NB).T
        )  # [NB, NBLK]
        yc = np.zeros(NSP, np.float32)
        yc[:NS] = y_train[c * NS : (c + 1) * NS]
        yb = yc.reshape(NBLK, NB).T.astype(bf)  # [NB, NBLK]
        y1z = np.zeros((NB, NBLK * 3 * 6), bf)
        for i in range(3):
            y1z[:, (3 * np.arange(NBLK) + i) * 6 + 2 * i] = yb
            y1z[:, (3 * np.arange(NBLK) + i) * 6 + 2 * i + 1] = 1.0
        in_maps.append(
            {"xa2": xa2, "ta2": ta2, "bias_t": bias_t, "y1z": y1z}
        )
    return in_maps


def _prep_inputs(x, X_train, y_train, mmdt="f32"):
    import ml_dtypes

    x = np.asarray(x, np.float32)
    X_train = np.asarray(X_train, np.float32)
    y_train = np.asarray(y_train, np.float32)

    xa = np.empty((K, B), np.float32)
    xa[:F] = x.T
    xa[F] = -0.5 * np.sum(x * x, axis=1)
    xa[F + 1] = 1.0

    in_maps = []
    for c in range(P):
        t = X_train[c * NS : (c + 1) * NS]
        ta = np.zeros((K, NSP), np.float32)
        ta[:F, :NS] = t.T
        ta[F, :] = 1.0
        ta[F + 1, :NS] = -0.5 * np.sum(t * t, axis=1)
        ta[F + 1, NS:] = -1e30  # pad columns get weight exp(-inf) = 0
        y1 = np.zeros((NB, 2 * NBLK), ml_dtypes.bfloat16)
        yc = np.zeros(NSP, np.float32)
        yc[:NS] = y_train[c * NS : (c + 1) * NS]
        yb = yc.reshape(NBLK, NB).T.astype(ml_dtypes.bfloat16)  # [NB, NBLK]
        y1[:, 0::2] = yb
        y1[:, 1::2] = 1.0
        y1z = np.zeros((NB, NBLK * 3 * 6), ml_dtypes.bfloat16)
        for i in range(3):
            y1z[:, (3 * np.arange(NBLK) + i) * 6 + 2 * i] = yb
            y1z[:, (3 * np.arange(NBLK) + i) * 6 + 2 * i + 1] = 1.0
        in_maps.append({"xa": xa, "ta": ta, "y1": y1, "y1z": y1z})
    if mmdt == "bf16":
        import ml_dtypes as _md

        for m in in_maps:
            m["xa"] = m["xa"].astype(_md.bfloat16)
            m["ta"] = np.clip(m["ta"], -3e38, 3e38).astype(_md.bfloat16)
    return in_maps


def kernel(x, X_train, y_train):
    from concourse.bass_utils import run_bass_kernel_spmd

    in_maps = _prep_inputs_v3(x, X_train, y_train)
    if "nc" not in _cache:
        _cache["nc"] = _build_v3()
    res = run_bass_kernel_spmd(_cache["nc"], in_maps, core_ids=list(range(P)))
    parts = np.stack([np.asarray(r["out"]) for r in res.results])  # [P, 2, B]
    tot = parts.sum(axis=0, dtype=np.float64)
    return (tot[0] / tot[1]).astype(np.float32)


# revision 17
# speedup vs baseline: 2.0745x; 1.0301x over previous
"""GRNN regressor on 8 TRN2 NeuronCores.

Math: w[b,n] = exp(-||x_b - t_n||^2 / 2); out[b] = (w@y)/(w@1).

Sharding (per the hint): X_train/y_train sharded over N across 8 cores,
queries replicated; each core computes partial [numerator; denominator]
of shape [2, 4096] and the host all-reduces (32KB/core) and divides.

Kernel design (see _build_v3):
- The query-norm factor exp(-||x_b||^2/2) cancels in the num/den ratio, so
  logit[n,b] = x_b . t_n - ||t_n||^2/2 is enough; the train-norm term rides
  the Exp activation's per-partition bias input, so the matmul contracts
  features only.
- Precision: split-bf16 (hi/lo) operands. With x_hi = bf16(x) duplicated and
  ta2 = [t_hi; t_lo] stacked, one K=128 bf16 matmul yields
  x_hi.(t_hi + t_lo) to ~2^-17 accuracy; bias is exact fp32. Measured rel
  err vs the fp32 reference: ~5e-3 (gate 2e-2). bf16 matmuls stream at
  1 cycle/row vs 4 for fp32 (and float32r is erratic on this toolchain).
- One [128,128] ta2 stationary serves rb=3 query blocks back-to-back
  (fewer weight reloads); Exp is batched over the 3 PSUM banks
  ([128,1536] per instruction) since Act throughput is ~1.1 ns/elem/part
  with a real per-instruction cost.
- The accumulating second matmul contracts 128 train points per step into
  a single packed [6,512] PSUM accumulator (y/ones placed at partition
  pair 2i via zero-padded [128,6] stationaries), so one bank serves all 3
  query blocks and start=True resets happen once per 98-block chain.
- Second matmuls are emitted one group late (software pipelining) so the
  PE never waits in-order behind the Exp it depends on; input DMAs are
  chunked so the first matmul starts after ~1/8 of ta2 has landed.
- Measured: ~0.4 ms/exec on hardware (Act-engine exp bound; PE ~320us,
  Act ~400us busy), vs ~2.5 ms for the fp32 version of the same pipeline.
"""

import numpy as np

B, F, N, P = 4096, 64, 100000, 8
NS = N // P            # 12500 train points per core
NB = 128               # train-point block (PSUM partitions)
NSP = ((NS + NB - 1) // NB) * NB   # 12544 padded
NBLK = NSP // NB       # 98
BBLK = 512             # query block (moving free dim / PSUM bank)
K = F + 2              # augmented contraction dim
G = 3                  # PSUM banks per activation batch

_cache = {}


def _build(reps=1, stages=("mm1", "act", "mm2"), mmdt="f32r", actmode="exp"):
    import concourse.bacc as bacc
    import concourse.mybir as mybir
    import concourse.tile as tile

    f32 = mybir.dt.float32
    f32r = mybir.dt.float32r
    bf16 = mybir.dt.bfloat16
    mdt = {"f32r": f32r, "f32": f32, "bf16": bf16}[mmdt]
    nc = bacc.Bacc("TRN2", target_bir_lowering=False, debug=False)
    xa_d = nc.dram_tensor("xa", [K, B], mdt, kind="ExternalInput")
    ta_d = nc.dram_tensor("ta", [K, NSP], mdt, kind="ExternalInput")
    y1_d = nc.dram_tensor("y1", [NB, 2 * NBLK], bf16, kind="ExternalInput")
    out_d = nc.dram_tensor("out", [2, B], f32, kind="ExternalOutput")

    with tile.TileContext(nc) as tc:
        with (
            tc.tile_pool(name="const", bufs=1) as cpool,
            tc.tile_pool(name="w", bufs=3) as wpool,
            tc.tile_pool(name="res", bufs=2) as rpool,
            tc.tile_pool(name="ps", bufs=2, space="PSUM") as spool,
            tc.tile_pool(name="pacc", bufs=2, space="PSUM") as apool,
        ):
            xa = cpool.tile([K, B], mdt)
            ta = cpool.tile([K, NSP], mdt)
            y1 = cpool.tile([NB, 2 * NBLK], bf16)
            nc.sync.dma_start(xa[:], xa_d[:])
            nc.sync.dma_start(ta[:], ta_d[:])
            nc.sync.dma_start(y1[:], y1_d[:])

            groups = []
            ni = 0
            while ni < NBLK:
                g = min(G, NBLK - ni)
                groups.append((ni, g))
                ni += g

            # Software pipeline: the accumulating matmul for group i is
            # emitted after group i+1's dist matmuls, so the Tensor engine
            # never stalls in-order behind an Exp it depends on.
            def emit_mm2(acc, w, ni, g):
                if "mm2" not in stages:
                    return
                for j in range(g):
                    nc.tensor.matmul(
                        acc[:],
                        y1[:, 2 * (ni + j) : 2 * (ni + j) + 2],
                        w[:, j * BBLK : (j + 1) * BBLK],
                        start=(ni + j == 0), stop=(ni + j == NBLK - 1),
                    )

            accs = {}
            pend = None
            for r in range(reps):
                for b in range(B // BBLK):
                    acc = apool.tile([2, BBLK], f32)
                    accs[(r, b)] = acc
                    xsl = xa[:, b * BBLK : (b + 1) * BBLK]
                    for ni, g in groups:
                        s = spool.tile([NB, G * BBLK], f32)
                        if "mm1" in stages:
                            for j in range(g):
                                nc.tensor.matmul(
                                    s[:, j * BBLK : (j + 1) * BBLK],
                                    ta[:, (ni + j) * NB : (ni + j + 1) * NB],
                                    xsl,
                                    start=True, stop=True,
                                )
                        w = wpool.tile([NB, G * BBLK], bf16)
                        if "act" in stages:
                            if actmode == "exp":
                                nc.scalar.activation(
                                    w[:, : g * BBLK], s[:, : g * BBLK],
                                    mybir.ActivationFunctionType.Exp,
                                )
                            else:
                                nc.vector.tensor_copy(
                                    w[:, : g * BBLK], s[:, : g * BBLK]
                                )
                        if pend is not None:
                            emit_mm2(*pend)
                        pend = (acc, w, ni, g)
                    # drain the previous block's finished accumulator
                    if b > 0 or r > 0:
                        pb = (r, b - 1) if b > 0 else (r - 1, B // BBLK - 1)
                        res = rpool.tile([2, BBLK], f32)
                        if "mm2" in stages:
                            nc.vector.tensor_copy(res[:], accs.pop(pb)[:])
                        else:
                            accs.pop(pb)
                            nc.vector.memset(res[:], 0.0)
                        nc.sync.dma_start(
                            out_d[:, pb[1] * BBLK : (pb[1] + 1) * BBLK], res[:]
                        )
            emit_mm2(*pend)
            pend = None
            pb = (reps - 1, B // BBLK - 1)
            res = rpool.tile([2, BBLK], f32)
            if "mm2" in stages:
                nc.vector.tensor_copy(res[:], accs.pop(pb)[:])
            else:
                accs.pop(pb)
                nc.vector.memset(res[:], 0.0)
            nc.sync.dma_start(out_d[:, pb[1] * BBLK : (pb[1] + 1) * BBLK], res[:])

    nc.compile()
    return nc


def _build_samestat(reps=1, rb=3, mmdt="f32r"):
    """Stationary-reuse variant: one ta-block stationary serves `rb`
    consecutive query blocks; per-super accumulators are packed into a
    single PSUM bank as [2*rb, 512] via zero-padded [128, 2*rb] y
    stationaries."""
    import concourse.bacc as bacc
    import concourse.mybir as mybir
    import concourse.tile as tile

    f32 = mybir.dt.float32
    f32r = mybir.dt.float32r
    bf16 = mybir.dt.bfloat16
    mdt = {"f32r": f32r, "f32": f32, "bf16": bf16}[mmdt]
    nc = bacc.Bacc("TRN2", target_bir_lowering=False, debug=False)
    xa_d = nc.dram_tensor("xa", [K, B], mdt, kind="ExternalInput")
    ta_d = nc.dram_tensor("ta", [K, NSP], mdt, kind="ExternalInput")
    y1z_d = nc.dram_tensor(
        "y1z", [NB, NBLK * 3 * 6], bf16, kind="ExternalInput"
    )
    out_d = nc.dram_tensor("out", [2, B], f32, kind="ExternalOutput")

    nb = B // BBLK
    supers = [list(range(i, min(i + rb, nb))) for i in range(0, nb, rb)]

    with tile.TileContext(nc) as tc:
        with (
            tc.tile_pool(name="const", bufs=1) as cpool,
            tc.tile_pool(name="w", bufs=3) as wpool,
            tc.tile_pool(name="res", bufs=2) as rpool,
            tc.tile_pool(name="ps", bufs=2, space="PSUM") as spool,
            tc.tile_pool(name="pacc", bufs=2, space="PSUM") as apool,
        ):
            xa = cpool.tile([K, B], mdt)
            ta = cpool.tile([K, NSP], mdt)
            y1z = cpool.tile([NB, NBLK * 3 * 6], bf16)
            nc.sync.dma_start(xa[:], xa_d[:])
            nc.sync.dma_start(ta[:], ta_d[:])
            nc.sync.dma_start(y1z[:], y1z_d[:])

            def emit_mm2(acc, w, ni, blocks):
                for i in range(len(blocks)):
                    c0 = (ni * 3 + i) * 6
                    nc.tensor.matmul(
                        acc[:],
                        y1z[:, c0 : c0 + 6],
                        w[:, i * BBLK : (i + 1) * BBLK],
                        start=(ni == 0 and i == 0),
                        stop=(ni == NBLK - 1 and i == len(blocks) - 1),
                    )

            def drain(acc, blocks):
                res = rpool.tile([6, BBLK], f32)
                nc.vector.tensor_copy(res[:], acc[:])
                for i, b in enumerate(blocks):
                    nc.sync.dma_start(
                        out_d[:, b * BBLK : (b + 1) * BBLK],
                        res[2 * i : 2 * i + 2, :],
                    )

            pend = None
            done = None
            for r in range(reps):
                for blocks in supers:
                    acc = apool.tile([6, BBLK], f32)
                    for ni in range(NBLK):
                        s = spool.tile([NB, 3 * BBLK], f32)
                        for i, b in enumerate(blocks):
                            nc.tensor.matmul(
                                s[:, i * BBLK : (i + 1) * BBLK],
                                ta[:, ni * NB : (ni + 1) * NB],
                                xa[:, b * BBLK : (b + 1) * BBLK],
                                start=True, stop=True,
                            )
                        w = wpool.tile([NB, 3 * BBLK], bf16)
                        nc.scalar.activation(
                            w[:, : len(blocks) * BBLK],
                            s[:, : len(blocks) * BBLK],
                            mybir.ActivationFunctionType.Exp,
                        )
                        if pend is not None:
                            emit_mm2(*pend)
                            if done is not None:
                                drain(*done)
                                done = None
                        pend = (acc, w, ni, blocks)
                    done = (acc, blocks)
            emit_mm2(*pend)
            drain(*done)

    nc.compile()
    return nc


def _build_v3(reps=1, rb=3, use_bias=True, actsplit=False):
    """Split-bf16 kernel: logit[n,b] = x_hi.(t_hi+t_lo) + bias_n, with
    bias_n = -||t_n||^2/2 folded into the Exp activation's per-partition
    bias; the query-norm factor cancels in numerator/denominator. One
    K=128 bf16 matmul per (train-block, query-block), stationary reused
    across `rb` query blocks; packed [6,512] PSUM accumulators."""
    import concourse.bacc as bacc
    import concourse.mybir as mybir
    import concourse.tile as tile

    f32 = mybir.dt.float32
    bf16 = mybir.dt.bfloat16
    nc = bacc.Bacc("TRN2", target_bir_lowering=False, debug=False)
    xa_d = nc.dram_tensor("xa2", [NB, B], bf16, kind="ExternalInput")
    ta_d = nc.dram_tensor("ta2", [NB, NSP], bf16, kind="ExternalInput")
    bi_d = nc.dram_tensor("bias_t", [NB, NBLK], f32, kind="ExternalInput")
    y1z_d = nc.dram_tensor(
        "y1z", [NB, NBLK * 3 * 6], bf16, kind="ExternalInput"
    )
    out_d = nc.dram_tensor("out", [2, B], f32, kind="ExternalOutput")

    nb = B // BBLK
    supers = [list(range(i, min(i + rb, nb))) for i in range(0, nb, rb)]

    with tile.TileContext(nc) as tc:
        with (
            tc.tile_pool(name="const", bufs=1) as cpool,
            tc.tile_pool(name="w", bufs=3) as wpool,
            tc.tile_pool(name="res", bufs=2) as rpool,
            tc.tile_pool(name="ps", bufs=2, space="PSUM") as spool,
            tc.tile_pool(name="pacc", bufs=2, space="PSUM") as apool,
        ):
            xa = cpool.tile([NB, B], bf16)
            ta = cpool.tile([NB, NSP], bf16)
            bi = cpool.tile([NB, NBLK], f32)
            y1z = cpool.tile([NB, NBLK * 3 * 6], bf16)
            nc.sync.dma_start(bi[:], bi_d[:])
            for c in range(0, B, BBLK):
                nc.sync.dma_start(
                    xa[:, c : c + BBLK], xa_d[:, c : c + BBLK]
                )
            for c in range(0, NSP, 8 * NB):
                e = min(c + 8 * NB, NSP)
                nc.sync.dma_start(ta[:, c:e], ta_d[:, c:e])
            zc = NBLK * 3 * 6 // 4
            for c in range(0, NBLK * 3 * 6, zc):
                nc.sync.dma_start(y1z[:, c : c + zc], y1z_d[:, c : c + zc])

            def emit_mm2(acc, w, ni, blocks):
                for i in range(len(blocks)):
                    c0 = (ni * 3 + i) * 6
                    nc.tensor.matmul(
                        acc[:],
                        y1z[:, c0 : c0 + 6],
                        w[:, i * BBLK : (i + 1) * BBLK],
                        start=(ni == 0 and i == 0),
                        stop=(ni == NBLK - 1 and i == len(blocks) - 1),
                    )

            def drain(acc, blocks):
                res = rpool.tile([6, BBLK], f32)
                nc.vector.tensor_copy(res[:], acc[:])
                for i, b in enumerate(blocks):
                    nc.sync.dma_start(
                        out_d[:, b * BBLK : (b + 1) * BBLK],
                        res[2 * i : 2 * i + 2, :],
                    )

            pend = None
            done = None
            for r in range(reps):
                for blocks in supers:
                    acc = apool.tile([6, BBLK], f32)
                    for ni in range(NBLK):
                        s = spool.tile([NB, 3 * BBLK], f32)
                        for i, b in enumerate(blocks):
                            nc.tensor.matmul(
                                s[:, i * BBLK : (i + 1) * BBLK],
                                ta[:, ni * NB : (ni + 1) * NB],
                                xa[:, b * BBLK : (b + 1) * BBLK],
                                start=True, stop=True,
                            )
                        w = wpool.tile([NB, 3 * BBLK], bf16)
                        if actsplit:
                            for i in range(len(blocks)):
                                nc.scalar.activation(
                                    w[:, i * BBLK : (i + 1) * BBLK],
                                    s[:, i * BBLK : (i + 1) * BBLK],
                                    mybir.ActivationFunctionType.Exp,
                                    bias=bi[:, ni : ni + 1] if use_bias else 0.0,
                                )
                        else:
                            nc.scalar.activation(
                                w[:, : len(blocks) * BBLK],
                                s[:, : len(blocks) * BBLK],
                                mybir.ActivationFunctionType.Exp,
                                bias=bi[:, ni : ni + 1] if use_bias else 0.0,
                            )
                        if pend is not None:
                            emit_mm2(*pend)
                            if done is not None:
                                drain(*done)
                                done = None
                        pend = (acc, w, ni, blocks)
                    done = (acc, blocks)
            emit_mm2(*pend)
            drain(*done)

    nc.compile()
    return nc


def _prep_inputs_v3(x, X_train, y_train):
    import ml_dtypes

    bf = ml_dtypes.bfloat16
    x = np.asarray(x, np.float32)
    X_train = np.asarray(X_train, np.float32)
    y_train = np.asarray(y_train, np.float32)

    x_hi = x.astype(bf).astype(np.float32)
    xa2 = np.empty((NB, B), bf)
    xa2[:F] = x_hi.T
    xa2[F:] = x_hi.T

    in_maps = []
    for c in range(P):
        t = X_train[c * NS : (c + 1) * NS]
        t_hi = t.astype(bf).astype(np.float32)
        t_lo = (t - t_hi).astype(bf).astype(np.float32)
        t2 = t_hi + t_lo
        ta2 = np.zeros((NB, NSP), bf)
        ta2[:F, :NS] = t_hi.T
        ta2[F:, :NS] = t_lo.T
        bias_t = np.full(NSP, -1e30, np.float32)
        bias_t[:NS] = -0.5 * np.sum(t2 * t2, axis=1)
        bias_t = np.ascontiguousarray(
            bias_t.reshape(NBLK, NB).T
        )  # [NB, NBLK]
        yc = np.zeros(NSP, np.float32)
        yc[:NS] = y_train[c * NS : (c + 1) * NS]
        yb = yc.reshape(NBLK, NB).T.astype(bf)  # [NB, NBLK]
        y1z = np.zeros((NB, NBLK * 3 * 6), bf)
        for i in range(3):
            y1z[:, (3 * np.arange(NBLK) + i) * 6 + 2 * i] = yb
            y1z[:, (3 * np.arange(NBLK) + i) * 6 + 2 * i + 1] = 1.0
        in_maps.append(
            {"xa2": xa2, "ta2": ta2, "bias_t": bias_t, "y1z": y1z}
        )
    return in_maps


def _prep_inputs(x, X_train, y_train, mmdt="f32"):
    import ml_dtypes

    x = np.asarray(x, np.float32)
    X_train = np.asarray(X_train, np.float32)
    y_train = np.asarray(y_train, np.float32)

    xa = np.empty((K, B), np.float32)
    xa[:F] = x.T
    xa[F] = -0.5 * np.sum(x * x, axis=1)
    xa[F + 1] = 1.0

    in_maps = []
    for c in range(P):
        t = X_train[c * NS : (c + 1) * NS]
        ta = np.zeros((K, NSP), np.float32)
        ta[:F, :NS] = t.T
        ta[F, :] = 1.0
        ta[F + 1, :NS] = -0.5 * np.sum(t * t, axis=1)
        ta[F + 1, NS:] = -1e30  # pad columns get weight exp(-inf) = 0
        y1 = np.zeros((NB, 2 * NBLK), ml_dtypes.bfloat16)
        yc = np.zeros(NSP, np.float32)
        yc[:NS] = y_train[c * NS : (c + 1) * NS]
        yb = yc.reshape(NBLK, NB).T.astype(ml_dtypes.bfloat16)  # [NB, NBLK]
        y1[:, 0::2] = yb
        y1[:, 1::2] = 1.0
        y1z = np.zeros((NB, NBLK * 3 * 6), ml_dtypes.bfloat16)
        for i in range(3):
            y1z[:, (3 * np.arange(NBLK) + i) * 6 + 2 * i] = yb
            y1z[:, (3 * np.arange(NBLK) + i) * 6 + 2 * i + 1] = 1.0
        in_maps.append({"xa": xa, "ta": ta, "y1": y1, "y1z": y1z})
    if mmdt == "bf16":
        import ml_dtypes as _md

        for m in in_maps:
            m["xa"] = m["xa"].astype(_md.bfloat16)
            m["ta"] = np.clip(m["ta"], -3e38, 3e38).astype(_md.bfloat16)
    return in_maps


def kernel(x, X_train, y_train):
    from concourse.bass_utils import run_bass_kernel_spmd

    in_maps = _prep_inputs_v3(x, X_train, y_train)
    if "nc" not in _cache:
        _cache["nc"] = _build_v3()
    res = run_bass_kernel_spmd(_cache["nc"], in_maps, core_ids=list(range(P)))
    parts = np.stack([np.asarray(r["out"]) for r in res.results])  # [P, 2, B]
    tot = parts.sum(axis=0, dtype=np.float64)
    return (tot[0] / tot[1]).astype(np.float32)


# revision 20
# speedup vs baseline: 2.1827x; 1.0522x over previous
"""GRNN regressor on 8 TRN2 NeuronCores.

Math: w[b,n] = exp(-||x_b - t_n||^2 / 2); out[b] = (w@y)/(w@1).

Sharding (per the hint): X_train/y_train sharded over N across 8 cores,
queries replicated; each core computes partial [numerator; denominator]
of shape [2, 4096] and the host all-reduces (32KB/core) and divides.

Kernel design (see _build_v3):
- The query-norm factor exp(-||x_b||^2/2) cancels in the num/den ratio, so
  logit[n,b] = x_b . t_n - ||t_n||^2/2 is enough; the train-norm term rides
  the Exp activation's per-partition bias input, so the matmul contracts
  features only.
- Precision: split-bf16 (hi/lo) operands. With x_hi = bf16(x) duplicated and
  ta2 = [t_hi; t_lo] stacked, one K=128 bf16 matmul yields
  x_hi.(t_hi + t_lo) to ~2^-17 accuracy; bias is exact fp32. Measured rel
  err vs the fp32 reference: ~5e-3 (gate 2e-2). bf16 matmuls stream at
  1 cycle/row vs 4 for fp32 (and float32r is erratic on this toolchain).
- One [128,128] ta2 stationary serves rb=3 query blocks back-to-back
  (fewer weight reloads); Exp is batched over the 3 PSUM banks
  ([128,1536] per instruction) since Act throughput is ~1.1 ns/elem/part
  with a real per-instruction cost.
- The accumulating second matmul contracts 128 train points per step into
  a single packed [6,512] PSUM accumulator (y/ones placed at partition
  pair 2i via zero-padded [128,6] stationaries), so one bank serves all 3
  query blocks and start=True resets happen once per 98-block chain.
- Second matmuls are emitted one group late (software pipelining) so the
  PE never waits in-order behind the Exp it depends on; input DMAs are
  chunked so the first matmul starts after ~1/8 of ta2 has landed.
- Measured: ~0.4 ms/exec on hardware (Act-engine exp bound; PE ~320us,
  Act ~400us busy), vs ~2.5 ms for the fp32 version of the same pipeline.
"""

import numpy as np

B, F, N, P = 4096, 64, 100000, 8
NS = N // P            # 12500 train points per core
NB = 128               # train-point block (PSUM partitions)
NSP = ((NS + NB - 1) // NB) * NB   # 12544 padded
NBLK = NSP // NB       # 98
BBLK = 512             # query block (moving free dim / PSUM bank)
K = F + 2              # augmented contraction dim
G = 3                  # PSUM banks per activation batch

_cache = {}


def _build(reps=1, stages=("mm1", "act", "mm2"), mmdt="f32r", actmode="exp"):
    import concourse.bacc as bacc
    import concourse.mybir as mybir
    import concourse.tile as tile

    f32 = mybir.dt.float32
    f32r = mybir.dt.float32r
    bf16 = mybir.dt.bfloat16
    mdt = {"f32r": f32r, "f32": f32, "bf16": bf16}[mmdt]
    nc = bacc.Bacc("TRN2", target_bir_lowering=False, debug=False)
    xa_d = nc.dram_tensor("xa", [K, B], mdt, kind="ExternalInput")
    ta_d = nc.dram_tensor("ta", [K, NSP], mdt, kind="ExternalInput")
    y1_d = nc.dram_tensor("y1", [NB, 2 * NBLK], bf16, kind="ExternalInput")
    out_d = nc.dram_tensor("out", [2, B], f32, kind="ExternalOutput")

    with tile.TileContext(nc) as tc:
        with (
            tc.tile_pool(name="const", bufs=1) as cpool,
            tc.tile_pool(name="w", bufs=3) as wpool,
            tc.tile_pool(name="res", bufs=2) as rpool,
            tc.tile_pool(name="ps", bufs=2, space="PSUM") as spool,
            tc.tile_pool(name="pacc", bufs=2, space="PSUM") as apool,
        ):
            xa = cpool.tile([K, B], mdt)
            ta = cpool.tile([K, NSP], mdt)
            y1 = cpool.tile([NB, 2 * NBLK], bf16)
            nc.sync.dma_start(xa[:], xa_d[:])
            nc.sync.dma_start(ta[:], ta_d[:])
            nc.sync.dma_start(y1[:], y1_d[:])

            groups = []
            ni = 0
            while ni < NBLK:
                g = min(G, NBLK - ni)
                groups.append((ni, g))
                ni += g

            # Software pipeline: the accumulating matmul for group i is
            # emitted after group i+1's dist matmuls, so the Tensor engine
            # never stalls in-order behind an Exp it depends on.
            def emit_mm2(acc, w, ni, g):
                if "mm2" not in stages:
                    return
                for j in range(g):
                    nc.tensor.matmul(
                        acc[:],
                        y1[:, 2 * (ni + j) : 2 * (ni + j) + 2],
                        w[:, j * BBLK : (j + 1) * BBLK],
                        start=(ni + j == 0), stop=(ni + j == NBLK - 1),
                    )

            accs = {}
            pend = None
            for r in range(reps):
                for b in range(B // BBLK):
                    acc = apool.tile([2, BBLK], f32)
                    accs[(r, b)] = acc
                    xsl = xa[:, b * BBLK : (b + 1) * BBLK]
                    for ni, g in groups:
                        s = spool.tile([NB, G * BBLK], f32)
                        if "mm1" in stages:
                            for j in range(g):
                                nc.tensor.matmul(
                                    s[:, j * BBLK : (j + 1) * BBLK],
                                    ta[:, (ni + j) * NB : (ni + j + 1) * NB],
                                    xsl,
                                    start=True, stop=True,
                                )
                        w = wpool.tile([NB, G * BBLK], bf16)
                        if "act" in stages:
                            if actmode == "exp":
                                nc.scalar.activation(
                                    w[:, : g * BBLK], s[:, : g * BBLK],
                                    mybir.ActivationFunctionType.Exp,
                                )
                            else:
                                nc.vector.tensor_copy(
                                    w[:, : g * BBLK], s[:, : g * BBLK]
                                )
                        if pend is not None:
                            emit_mm2(*pend)
                        pend = (acc, w, ni, g)
                    # drain the previous block's finished accumulator
                    if b > 0 or r > 0:
                        pb = (r, b - 1) if b > 0 else (r - 1, B // BBLK - 1)
                        res = rpool.tile([2, BBLK], f32)
                        if "mm2" in stages:
                            nc.vector.tensor_copy(res[:], accs.pop(pb)[:])
                        else:
                            accs.pop(pb)
                            nc.vector.memset(res[:], 0.0)
                        nc.sync.dma_start(
                            out_d[:, pb[1] * BBLK : (pb[1] + 1) * BBLK], res[:]
                        )
            emit_mm2(*pend)
            pend = None
            pb = (reps - 1, B // BBLK - 1)
            res = rpool.tile([2, BBLK], f32)
            if "mm2" in stages:
                nc.vector.tensor_copy(res[:], accs.pop(pb)[:])
            else:
                accs.pop(pb)
                nc.vector.memset(res[:], 0.0)
            nc.sync.dma_start(out_d[:, pb[1] * BBLK : (pb[1] + 1) * BBLK], res[:])

    nc.compile()
    return nc


def _build_samestat(reps=1, rb=3, mmdt="f32r"):
    """Stationary-reuse variant: one ta-block stationary serves `rb`
    consecutive query blocks; per-super accumulators are packed into a
    single PSUM bank as [2*rb, 512] via zero-padded [128, 2*rb] y
    stationaries."""
    import concourse.bacc as bacc
    import concourse.mybir as mybir
    import concourse.tile as tile

    f32 = mybir.dt.float32
    f32r = mybir.dt.float32r
    bf16 = mybir.dt.bfloat16
    mdt = {"f32r": f32r, "f32": f32, "bf16": bf16}[mmdt]
    nc = bacc.Bacc("TRN2", target_bir_lowering=False, debug=False)
    xa_d = nc.dram_tensor("xa", [K, B], mdt, kind="ExternalInput")
    ta_d = nc.dram_tensor("ta", [K, NSP], mdt, kind="ExternalInput")
    y1z_d = nc.dram_tensor(
        "y1z", [NB, NBLK * 3 * 6], bf16, kind="ExternalInput"
    )
    out_d = nc.dram_tensor("out", [2, B], f32, kind="ExternalOutput")

    nb = B // BBLK
    supers = [list(range(i, min(i + rb, nb))) for i in range(0, nb, rb)]

    with tile.TileContext(nc) as tc:
        with (
            tc.tile_pool(name="const", bufs=1) as cpool,
            tc.tile_pool(name="w", bufs=3) as wpool,
            tc.tile_pool(name="res", bufs=2) as rpool,
            tc.tile_pool(name="ps", bufs=2, space="PSUM") as spool,
            tc.tile_pool(name="pacc", bufs=2, space="PSUM") as apool,
        ):
            xa = cpool.tile([K, B], mdt)
            ta = cpool.tile([K, NSP], mdt)
            y1z = cpool.tile([NB, NBLK * 3 * 6], bf16)
            nc.sync.dma_start(xa[:], xa_d[:])
            nc.sync.dma_start(ta[:], ta_d[:])
            nc.sync.dma_start(y1z[:], y1z_d[:])

            def emit_mm2(acc, w, ni, blocks):
                for i in range(len(blocks)):
                    c0 = (ni * 3 + i) * 6
                    nc.tensor.matmul(
                        acc[:],
                        y1z[:, c0 : c0 + 6],
                        w[:, i * BBLK : (i + 1) * BBLK],
                        start=(ni == 0 and i == 0),
                        stop=(ni == NBLK - 1 and i == len(blocks) - 1),
                    )

            def drain(acc, blocks):
                res = rpool.tile([6, BBLK], f32)
                nc.vector.tensor_copy(res[:], acc[:])
                for i, b in enumerate(blocks):
                    nc.sync.dma_start(
                        out_d[:, b * BBLK : (b + 1) * BBLK],
                        res[2 * i : 2 * i + 2, :],
                    )

            pends = []
            for r in range(reps):
                for blocks in supers:
                    acc = apool.tile([6, BBLK], f32)
                    for ni in range(NBLK):
                        s = spool.tile([NB, 3 * BBLK], f32)
                        for i, b in enumerate(blocks):
                            nc.tensor.matmul(
                                s[:, i * BBLK : (i + 1) * BBLK],
                                ta[:, ni * NB : (ni + 1) * NB],
                                xa[:, b * BBLK : (b + 1) * BBLK],
                                start=True, stop=True,
                            )
                        w = wpool.tile([NB, 3 * BBLK], bf16)
                        nc.scalar.activation(
                            w[:, : len(blocks) * BBLK],
                            s[:, : len(blocks) * BBLK],
                            mybir.ActivationFunctionType.Exp,
                        )
                        pends.append((acc, w, ni, blocks))
                        if len(pends) > depth:
                            it = pends.pop(0)
                            emit_mm2(*it)
                            if it[2] == NBLK - 1:
                                drain(it[0], it[3])
            while pends:
                it = pends.pop(0)
                emit_mm2(*it)
                if it[2] == NBLK - 1:
                    drain(it[0], it[3])

    nc.compile()
    return nc


def _build_v3(reps=1, rb=3, use_bias=True, actsplit=False, wbufs=5, depth=3):
    """Split-bf16 kernel: logit[n,b] = x_hi.(t_hi+t_lo) + bias_n, with
    bias_n = -||t_n||^2/2 folded into the Exp activation's per-partition
    bias; the query-norm factor cancels in numerator/denominator. One
    K=128 bf16 matmul per (train-block, query-block), stationary reused
    across `rb` query blocks; packed [6,512] PSUM accumulators."""
    import concourse.bacc as bacc
    import concourse.mybir as mybir
    import concourse.tile as tile

    f32 = mybir.dt.float32
    bf16 = mybir.dt.bfloat16
    nc = bacc.Bacc("TRN2", target_bir_lowering=False, debug=False)
    xa_d = nc.dram_tensor("xa2", [NB, B], bf16, kind="ExternalInput")
    ta_d = nc.dram_tensor("ta2", [NB, NSP], bf16, kind="ExternalInput")
    bi_d = nc.dram_tensor("bias_t", [NB, NBLK], f32, kind="ExternalInput")
    y1z_d = nc.dram_tensor(
        "y1z", [NB, NBLK * 3 * 6], bf16, kind="ExternalInput"
    )
    out_d = nc.dram_tensor("out", [2, B], f32, kind="ExternalOutput")

    nb = B // BBLK
    supers = [list(range(i, min(i + rb, nb))) for i in range(0, nb, rb)]

    with tile.TileContext(nc) as tc:
        with (
            tc.tile_pool(name="const", bufs=1) as cpool,
            tc.tile_pool(name="w", bufs=wbufs) as wpool,
            tc.tile_pool(name="res", bufs=2) as rpool,
            tc.tile_pool(name="ps", bufs=2, space="PSUM") as spool,
            tc.tile_pool(name="pacc", bufs=2, space="PSUM") as apool,
        ):
            xa = cpool.tile([NB, B], bf16)
            ta = cpool.tile([NB, NSP], bf16)
            bi = cpool.tile([NB, NBLK], f32)
            y1z = cpool.tile([NB, NBLK * 3 * 6], bf16)
            nc.sync.dma_start(bi[:], bi_d[:])
            for c in range(0, B, BBLK):
                nc.sync.dma_start(
                    xa[:, c : c + BBLK], xa_d[:, c : c + BBLK]
                )
            for c in range(0, NSP, 8 * NB):
                e = min(c + 8 * NB, NSP)
                nc.sync.dma_start(ta[:, c:e], ta_d[:, c:e])
            zc = NBLK * 3 * 6 // 4
            for c in range(0, NBLK * 3 * 6, zc):
                nc.sync.dma_start(y1z[:, c : c + zc], y1z_d[:, c : c + zc])

            def emit_mm2(acc, w, ni, blocks):
                for i in range(len(blocks)):
                    c0 = (ni * 3 + i) * 6
                    nc.tensor.matmul(
                        acc[:],
                        y1z[:, c0 : c0 + 6],
                        w[:, i * BBLK : (i + 1) * BBLK],
                        start=(ni == 0 and i == 0),
                        stop=(ni == NBLK - 1 and i == len(blocks) - 1),
                    )

            def drain(acc, blocks):
                res = rpool.tile([6, BBLK], f32)
                nc.vector.tensor_copy(res[:], acc[:])
                for i, b in enumerate(blocks):
                    nc.sync.dma_start(
                        out_d[:, b * BBLK : (b + 1) * BBLK],
                        res[2 * i : 2 * i + 2, :],
                    )

            pends = []
            for r in range(reps):
                for blocks in supers:
                    acc = apool.tile([6, BBLK], f32)
                    for ni in range(NBLK):
                        s = spool.tile([NB, 3 * BBLK], f32)
                        for i, b in enumerate(blocks):
                            nc.tensor.matmul(
                                s[:, i * BBLK : (i + 1) * BBLK],
                                ta[:, ni * NB : (ni + 1) * NB],
                                xa[:, b * BBLK : (b + 1) * BBLK],
                                start=True, stop=True,
                            )
                        w = wpool.tile([NB, 3 * BBLK], bf16)
                        if actsplit:
                            for i in range(len(blocks)):
                                nc.scalar.activation(
                                    w[:, i * BBLK : (i + 1) * BBLK],
                                    s[:, i * BBLK : (i + 1) * BBLK],
                                    mybir.ActivationFunctionType.Exp,
                                    bias=bi[:, ni : ni + 1] if use_bias else 0.0,
                                )
                        else:
                            nc.scalar.activation(
                                w[:, : len(blocks) * BBLK],
                                s[:, : len(blocks) * BBLK],
                                mybir.ActivationFunctionType.Exp,
                                bias=bi[:, ni : ni + 1] if use_bias else 0.0,
                            )
                        pends.append((acc, w, ni, blocks))
                        if len(pends) > depth:
                            it = pends.pop(0)
                            emit_mm2(*it)
                            if it[2] == NBLK - 1:
                                drain(it[0], it[3])
            while pends:
                it = pends.pop(0)
                emit_mm2(*it)
                if it[2] == NBLK - 1:
                    drain(it[0], it[3])

    nc.compile()
    return nc


def _prep_inputs_v3(x, X_train, y_train):
    import ml_dtypes

    bf = ml_dtypes.bfloat16
    x = np.asarray(x, np.float32)
    X_train = np.asarray(X_train, np.float32)
    y_train = np.asarray(y_train, np.float32)

    x_hi = x.astype(bf).astype(np.float32)
    xa2 = np.empty((NB, B), bf)
    xa2[:F] = x_hi.T
    xa2[F:] = x_hi.T

    in_maps = []
    for c in range(P):
        t = X_train[c * NS : (c + 1) * NS]
        t_hi = t.astype(bf).astype(np.float32)
        t_lo = (t - t_hi).astype(bf).astype(np.float32)
        t2 = t_hi + t_lo
        ta2 = np.zeros((NB, NSP), bf)
        ta2[:F, :NS] = t_hi.T
        ta2[F:, :NS] = t_lo.T
        bias_t = np.full(NSP, -1e30, np.float32)
        bias_t[:NS] = -0.5 * np.sum(t2 * t2, axis=1)
        bias_t = np.ascontiguousarray(
            bias_t.reshape(NBLK, NB).T
        )  # [NB, NBLK]
        yc = np.zeros(NSP, np.float32)
        yc[:NS] = y_train[c * NS : (c + 1) * NS]
        yb = yc.reshape(NBLK, NB).T.astype(bf)  # [NB, NBLK]
        y1z = np.zeros((NB, NBLK * 3 * 6), bf)
        for i in range(3):
            y1z[:, (3 * np.arange(NBLK) + i) * 6 + 2 * i] = yb
            y1z[:, (3 * np.arange(NBLK) + i) * 6 + 2 * i + 1] = 1.0
        in_maps.append(
            {"xa2": xa2, "ta2": ta2, "bias_t": bias_t, "y1z": y1z}
        )
    return in_maps


def _prep_inputs(x, X_train, y_train, mmdt="f32"):
    import ml_dtypes

    x = np.asarray(x, np.float32)
    X_train = np.asarray(X_train, np.float32)
    y_train = np.asarray(y_train, np.float32)

    xa = np.empty((K, B), np.float32)
    xa[:F] = x.T
    xa[F] = -0.5 * np.sum(x * x, axis=1)
    xa[F + 1] = 1.0

    in_maps = []
    for c in range(P):
        t = X_train[c * NS : (c + 1) * NS]
        ta = np.zeros((K, NSP), np.float32)
        ta[:F, :NS] = t.T
        ta[F, :] = 1.0
        ta[F + 1, :NS] = -0.5 * np.sum(t * t, axis=1)
        ta[F + 1, NS:] = -1e30  # pad columns get weight exp(-inf) = 0
        y1 = np.zeros((NB, 2 * NBLK), ml_dtypes.bfloat16)
        yc = np.zeros(NSP, np.float32)
        yc[:NS] = y_train[c * NS : (c + 1) * NS]
        yb = yc.reshape(NBLK, NB).T.astype(ml_dtypes.bfloat16)  # [NB, NBLK]
        y1[:, 0::2] = yb
        y1[:, 1::2] = 1.0
        y1z = np.zeros((NB, NBLK * 3 * 6), ml_dtypes.bfloat16)
        for i in range(3):
            y1z[:, (3 * np.arange(NBLK) + i) * 6 + 2 * i] = yb
            y1z[:, (3 * np.arange(NBLK) + i) * 6 + 2 * i + 1] = 1.0
        in_maps.append({"xa": xa, "ta": ta, "y1": y1, "y1z": y1z})
    if mmdt == "bf16":
        import ml_dtypes as _md

        for m in in_maps:
            m["xa"] = m["xa"].astype(_md.bfloat16)
            m["ta"] = np.clip(m["ta"], -3e38, 3e38).astype(_md.bfloat16)
    return in_maps


def kernel(x, X_train, y_train):
    from concourse.bass_utils import run_bass_kernel_spmd

    in_maps = _prep_inputs_v3(x, X_train, y_train)
    if "nc" not in _cache:
        _cache["nc"] = _build_v3()
    res = run_bass_kernel_spmd(_cache["nc"], in_maps, core_ids=list(range(P)))
    parts = np.stack([np.asarray(r["out"]) for r in res.results])  # [P, 2, B]
    tot = parts.sum(axis=0, dtype=np.float64)
    return (tot[0] / tot[1]).astype(np.float32)
